# revision 21
# baseline (speedup 1.0000x reference)
"""BlockwiseQuantLinear on 8 trn2 NeuronCores.

y = act_quant_dequant(x) @ (fp8_weight * block_scales).T
  x: [8192, 2048] f32, weight: [2048, 2048] fp8_e4m3fn (OCP), w_scale: [16, 16] f32
  out: [8192, 2048] f32

Strategy (data-parallel over tokens; hardcoded shapes):
  - Host: dequantize the static weight to fp16 (exact wrt reference up to fp16
    rounding), pre-transpose K-major as [wc, ki, j, n]; chunk DMAs land
    [128 ki, 4 kb, 2048 n] in SBUF with 16KB-contiguous rows. Shard x rows 8
    ways. Output stored fp16 on device, widened to f32 on host.
  - Device (per core, M_sh=1024): weights fully SBUF-resident (64KB/partition),
    loaded over both HWDGE rings in kb order (chunk wc is first needed at
    mm0_start + 3.4us*wc). Per 128-row x tile: 1MB load (sync ring);
    blockwise act quant per (1,128) k-block: absmax (DVE) -> 224/amax scale ->
    TRN fp8e4 cast (DVE, bit-matches the OCP e4m3fn reference quantization at
    half scale) -> fp16 dequant (GpSimd, off the critical DVE queue);
    DMA-xbar transpose per 1024-wide half, split across the two HWDGE rings.
  - Matmul stream: K-contiguous per m-tile — for kb in 16: for c in 4:
    psum[c] += xT[kb].T @ w[kb, c]. Stationary reused across the 4 n-chunk
    matmuls, 8 psum banks double-buffer across m-tiles, and the PE sees one
    long back-to-back stream so the HAM p-state holds 2.4GHz. Warm-up matmuls
    on the identity cover the ~3.4us p-state ramp during the initial
    load/quant latency.
  - Emission is software-pipelined TWO m-tiles ahead (back(mi) emitted after
    front(mi+2)). This is both a throughput and a correctness measure: the
    xbar-transpose completion semaphore is a FixedSemInc(+16) lane shared
    with later DMAs, so a consumer racing a just-issued transpose can observe
    its wait satisfied by another DMA's increments while descriptors are
    still landing (seen on hardware as a corrupted final m-tile when the PE
    caught up with the producer). With lag 2, every tile's transposes land a
    full matmul-stream (~14us) before the PE reads them.
  - Last tile stores per n-chunk right after each bank evict to shorten the
    tail; other tiles store one [128, 2048] row block.
  - Gather: concatenate the 8 row shards, astype(f32).
"""

import numpy as np
import ml_dtypes

import concourse.bass as bass
import concourse.mybir as mybir
import concourse.tile as tile
from concourse import bacc
from concourse.bass_utils import run_bass_kernel_spmd
from concourse.masks import make_identity

P = 128
M, K, N = 8192, 2048, 2048
NCORES = 8
M_SH = M // NCORES            # 1024 rows per core
MT = M_SH // P                # 8 m-tiles per core
KB = K // P                   # 16 k blocks
H = 2                         # halves per m-tile (quant/transpose granularity)
KBH = KB // H                 # 8 k blocks per half
KH_W = KBH * P                # 1024
NCH = 4                       # n chunks of 512
NC_W = N // NCH               # 512
WCH = 4                       # weight dma chunks of 4 kb each
EPS = 1e-12
N_WARM = 144                  # warm-up matmuls ([128,128] each)
LAG = 2                       # back(mi) emitted after front(mi+LAG)

_cache = {}


def _build():
    nc = bacc.Bacc(None, target_bir_lowering=False, num_swdge_queues=4)

    x_in = nc.dram_tensor("x_sh", [M_SH, K], mybir.dt.float32, kind="ExternalInput")
    # [wc, ki, j, n]: chunk wc holds k-blocks kb=4*wc+j, 16KB contiguous rows
    w_in = nc.dram_tensor(
        "wT", [WCH, P, KB // WCH, N], mybir.dt.float16, kind="ExternalInput"
    )
    y_out = nc.dram_tensor("y_sh", [M_SH, N], mybir.dt.float16, kind="ExternalOutput")

    with tile.TileContext(nc) as tc:
        with (
            tc.tile_pool(name="wpool", bufs=1) as wpool,
            tc.tile_pool(name="xpool", bufs=4) as xpool,
            tc.tile_pool(name="qpool", bufs=6) as qpool,
            tc.tile_pool(name="tpool", bufs=4) as tpool,
            tc.tile_pool(name="spool", bufs=4) as spool,
            tc.tile_pool(name="ypool", bufs=3) as ypool,
            tc.tile_pool(name="ps", bufs=2, space="PSUM") as ps,
        ):
            ident = spool.tile([P, P], mybir.dt.float16, name="ident", bufs=1)
            make_identity(nc, ident[:])

            wts = wpool.tile([P, KB, N], mybir.dt.float16, name="wts")

            def load_w(wc, eng):
                eng.dma_start(wts[:, bass.ts(wc, KB // WCH), :], w_in[wc])

            # w0/w1/w3 head the scalar ring (chunk wc first needed at
            # mm0_start + 3.4us*wc); w2 rides the sync ring after the first
            # two x loads.
            load_w(0, nc.scalar)
            load_w(1, nc.scalar)
            load_w(3, nc.scalar)

            # warm-up matmuls: keep the PE HAM activity window full while the
            # first x tile loads/quantizes, so real matmuls start at 2.4GHz.
            # Drawn from the psc0 tag so the 4 double-buffered chunk tags use
            # exactly the 8 PSUM banks (bufs are per-tag).
            warm_ps = ps.tile([P, NC_W], mybir.dt.float32, name="psc0", bufs=2)
            for _ in range(N_WARM):
                nc.tensor.matmul(
                    warm_ps[:, :P], ident[:], ident[:], start=True, stop=True
                )

            def quant(xg, t8, xdq, h):
                """Blockwise act-quant chain for half h: absmax per (1,128)
                block -> exact-match fp8 quantize (224 trick) -> fp16 dequant.
                All on DVE: GpSimd writes are retired before the DSPs flush,
                so a GpSimd dequant raced the xbar transpose reading xdq
                (seen on hw as corrupted late m-tiles)."""
                x3 = xg[:, bass.ts(h, KH_W)].rearrange(
                    "p (kb ki) -> p kb ki", kb=KBH
                )
                amax = spool.tile([P, KBH], mybir.dt.float32, name=f"amax{h}", bufs=4)
                nc.vector.tensor_reduce(
                    amax[:], x3, axis=mybir.AxisListType.X,
                    op=mybir.AluOpType.max, apply_absolute_value=True,
                )
                amaxp = spool.tile([P, KBH], mybir.dt.float32, name=f"amaxp{h}", bufs=4)
                nc.vector.tensor_scalar_max(amaxp[:], amax[:], EPS)
                rec = spool.tile([P, KBH], mybir.dt.float32, name=f"rec{h}", bufs=4)
                nc.vector.reciprocal(rec[:], amaxp[:])
                # the x224 / /224 scale muls run on ACT (Copy activation with
                # scale) to keep the DVE queue short
                inv2 = spool.tile([P, KBH], mybir.dt.float32, name=f"inv2_{h}", bufs=4)
                nc.scalar.activation(
                    inv2[:], rec[:], mybir.ActivationFunctionType.Copy, scale=224.0
                )
                s2 = spool.tile([P, KBH], mybir.dt.float32, name=f"s2_{h}", bufs=4)
                nc.scalar.activation(
                    s2[:], amaxp[:], mybir.ActivationFunctionType.Copy,
                    scale=1.0 / 224.0,
                )

                t83 = t8[:, bass.ts(h, KH_W)].rearrange("p (kb ki) -> p kb ki", kb=KBH)
                nc.vector.tensor_tensor(
                    t83, x3, inv2[:, :, None].to_broadcast([P, KBH, P]),
                    mybir.AluOpType.mult,
                )
                xdq3 = xdq[:, bass.ts(h, KH_W)].rearrange(
                    "p (kb ki) -> p kb ki", kb=KBH
                )
                # fp16 dequant on GpSimd: only affects the fp16 approximation
                # (not the reference-matching fp8 grid) and frees the DVE for
                # the next tile's reduce/quantize
                nc.gpsimd.tensor_tensor(
                    xdq3, t83, s2[:, :, None].to_broadcast([P, KBH, P]),
                    mybir.AluOpType.mult,
                )

            xgs = {}
            xTs = {}

            def load(mi):
                xg = xpool.tile([P, K], mybir.dt.float32, name="xg", bufs=4)
                nc.sync.dma_start(xg[:], x_in[bass.ts(mi, P), :])
                xgs[mi] = xg

            def front(mi):
                """Quant + transpose for m-tile mi."""
                xg = xgs.pop(mi)
                t8 = qpool.tile([P, K], mybir.dt.float8e4, name="t8", bufs=6)
                xdq = qpool.tile([P, K], mybir.dt.float16, name="xdq", bufs=6)
                xT = tpool.tile([P, KB, P], mybir.dt.float16, name="xT", bufs=4)
                for h in range(H):
                    quant(xg, t8, xdq, h)
                    # ALL transposes on the sync ring: two concurrent
                    # xbar transposes (one per HWDGE ring) interfere in the
                    # shared S2M xbar — seen on hw as the last source row of
                    # each 16-row xbar tile corrupted in whichever col-tiles
                    # overlapped the concurrency window. One ring = FIFO =
                    # never concurrent.
                    nc.sync.dma_start_transpose(
                        xT[:, bass.ts(h, KBH), :], xdq[:, bass.ts(h, KH_W)]
                    )
                xTs[mi] = xT

            def back(mi):
                """K-contiguous matmul stream + evict + store for m-tile mi."""
                xT = xTs.pop(mi)
                pss = [
                    ps.tile([P, NC_W], mybir.dt.float32, name=f"psc{c}", bufs=2)
                    for c in range(NCH)
                ]
                for kb in range(KB):
                    for c in range(NCH):
                        nc.tensor.matmul(
                            pss[c][:], xT[:, kb, :], wts[:, kb, bass.ts(c, NC_W)],
                            start=(kb == 0), stop=(kb == KB - 1),
                        )
                yt = ypool.tile([P, N], mybir.dt.float16, name="yt", bufs=3)
                if mi == MT - 1:
                    # shorten the tail: store each chunk right after its evict
                    for c in range(NCH):
                        nc.scalar.copy(yt[:, bass.ts(c, NC_W)], pss[c][:])
                        nc.scalar.dma_start(
                            y_out[bass.ts(mi, P), bass.ts(c, NC_W)],
                            yt[:, bass.ts(c, NC_W)],
                        )
                else:
                    # evicts explicitly on ACT: 'any' placed some on the DVE,
                    # whose queue is the producer critical path
                    for c in range(NCH):
                        nc.scalar.copy(yt[:, bass.ts(c, NC_W)], pss[c][:])
                    nc.scalar.dma_start(y_out[bass.ts(mi, P), :], yt[:])

            # software-pipelined emission, LAG tiles of slack between a
            # tile's transposes and its matmul stream
            load(0)
            load(1)
            load_w(2, nc.sync)
            for step in range(MT + LAG):
                if step < MT:
                    front(step)
                if step + 2 < MT:
                    load(step + 2)
                if step >= LAG:
                    back(step - LAG)

    nc.compile()
    return nc


def _prep_weight(weight: np.ndarray, w_scale: np.ndarray) -> np.ndarray:
    w_f32 = weight.astype(np.float32)                     # exact
    ws_full = np.repeat(np.repeat(w_scale.astype(np.float32), P, axis=0), P, axis=1)
    w_deq = (w_f32 * ws_full).astype(np.float16)          # [N, K]
    # w_deq.T[k, n]: k = (wc*4 + j)*128 + ki -> [wc, ki, j, n]
    wt = np.ascontiguousarray(
        w_deq.T.reshape(WCH, KB // WCH, P, N).transpose(0, 2, 1, 3)
    )
    return wt


def kernel(x: np.ndarray, weight: np.ndarray, w_scale: np.ndarray, _trace: bool = False):
    if "nc" not in _cache:
        _cache["nc"] = _build()
    nc = _cache["nc"]

    weight = np.asarray(weight)
    w_scale = np.asarray(w_scale, dtype=np.float32)
    wt = _prep_weight(weight, w_scale)
    x = np.ascontiguousarray(np.asarray(x), dtype=np.float32)

    in_maps = [
        {"x_sh": x[c * M_SH:(c + 1) * M_SH], "wT": wt}
        for c in range(NCORES)
    ]
    res = run_bass_kernel_spmd(
        nc, in_maps, core_ids=list(range(NCORES)),
        trace=_trace, trace_cores=list(range(NCORES)) if _trace else None,
    )
    y = np.concatenate(
        [res.results[c]["y_sh"] for c in range(NCORES)], axis=0
    ).astype(np.float32)
    if _trace:
        kernel.last_results = res
    return y
